# revision 3
# baseline (speedup 1.0000x reference)
"""Depthwise 3x3 CNN combo kernel for TRN2 (8 NeuronCores, channel-parallel).

Computes  out = relu(x*a0 + dwconv(x,w1)*a1 + dwconv(x,w2)*a2 + dwconv(x,w3)*a3)
for x [8, 256, 128, 128] f32 by folding everything into a single 9-tap
depthwise conv (conv is linear in the weights; the residual a0*x is the
center tap):  w_eff = a1*w1 + a2*w2 + a3*w3,  w_eff[:,1,1] += a0.

Sharding: CHANNELS across the 8 cores (32 channels x 8 batch images per
core).  Per-core layout puts image ROWS on the partitions:

  x tile  [y=128, (c, b, w=132)]   (w padded 2 left / 2 right with zeros,
                                    host-prepadded so DMA runs are 2112B)

The vertical 3-tap conv then becomes a matmul over the partition (row)
dim with a TRIDIAGONAL stationary matrix T_dx[yi, yo] = w_eff[c, yi-yo+1, dx]
(one matrix per channel and horizontal offset dx).  Each streamed rhs
column picks up all 3 vertical taps at once (384 useful MACs/cycle vs
128 for a diagonal matmul), so the full 9-tap conv needs only 3
accumulating matmuls per psum tile:

  psum[yo, (b, t)] += sum_yi T_dx[yi, yo] * xt[yi, (b, t + dx)]   dx = 0..2

with the dx shift handled as a free-dim offset into the padded row.
Row-boundary zero padding falls out of the band truncating at the matrix
edge.  PSUM accumulates in f32; relu+downcast to bf16 runs on ScalarE
(2/3) and DVE (1/3); host upcasts.

The tridiagonal matrices are built on DVE from 3 host-supplied one-hot
diagonal masks and per-(c,dx,diag) scalars (3 small ops per matrix), so
the only HBM traffic is x in (8.7MB) and y out (8.4MB) per core.
"""

import numpy as np

import concourse.bacc as bacc
import concourse.mybir as mybir
from concourse import bass_utils
from concourse.tile import TileContext

# Problem constants (hardcoded per contract).
B = 8
C = 256
H = 128
W = 128
NCORES = 8

CPC = C // NCORES   # channels per core
P = 128             # partitions (= H rows)
WP = W + 4          # padded row width (2 zero cols each side)
GROUPS = [(0, 3), (3, 3), (6, 2)]  # batch-image groups per psum bank

F32 = mybir.dt.float32
BF16 = mybir.dt.bfloat16

# "host": DMA full tridiag matrices from HBM.  "dve": build them on-chip
# from diagonal masks + per-channel scalars (less DMA, more DVE).
W_MODE = "dve"


def build_tile_kernel(tc, y_ap, x_ap, w_ap, dmask_ap=None, srep_ap=None):
    nc = tc.nc
    relu = mybir.ActivationFunctionType.Relu
    mult, add = mybir.AluOpType.mult, mybir.AluOpType.add

    with (
        tc.tile_pool(name="xpool", bufs=8) as xpool,
        tc.tile_pool(name="wpool", bufs=1) as wpool,
        tc.tile_pool(name="psum", bufs=6, space="PSUM") as psum_pool,
        tc.tile_pool(name="opool", bufs=8) as opool,
    ):
        # Input chunks of 4 channels: [y, c4, b, WP], one 8448B run per
        # partition.  All 8 chunks stay live (67KB/partition total).
        xts = []
        for g in range(CPC // 4):
            xt = xpool.tile([P, 4, B, WP], BF16, tag="xt")
            nc.sync.dma_start(xt[:], x_ap[:, 4 * g : 4 * g + 4])
            xts.append(xt)

        wt = wpool.tile([P, CPC, 3, P], BF16)
        if W_MODE == "host":
            # Full tridiag matrices from HBM (3MB): 4 chunks of 8 channels.
            for g in range(4):
                nc.scalar.dma_start(
                    wt[:, 8 * g : 8 * g + 8], w_ap[:, 8 * g : 8 * g + 8]
                )
        else:
            # Diagonal one-hot masks [y, diag, yo] + replicated scalars
            # [y, c, dx, diag]; build each T_dx as 1 TS + 2 STT on DVE.
            dmask = wpool.tile([P, 3, P], BF16)
            nc.scalar.dma_start(dmask[:], dmask_ap)
            srep = wpool.tile([P, CPC, 3, 3], F32)
            nc.scalar.dma_start(srep[:], srep_ap)
            for c in range(CPC):
                for dx in range(3):
                    dst = wt[:, c, dx, :]
                    nc.vector.tensor_scalar_mul(
                        dst, dmask[:, 1, :], srep[:, c, dx, 1:2]
                    )
                    nc.vector.scalar_tensor_tensor(
                        dst, dmask[:, 0, :], srep[:, c, dx, 0:1], dst,
                        mult, add,
                    )
                    nc.vector.scalar_tensor_tensor(
                        dst, dmask[:, 2, :], srep[:, c, dx, 2:3], dst,
                        mult, add,
                    )

        for c in range(CPC):
            xt = xts[c // 4]
            cc = c % 4
            ot = opool.tile([P, B, W], BF16)
            psums = [
                psum_pool.tile([P, nb * (W + 2)], F32, name="ps", tag="ps")
                for _, nb in GROUPS
            ]
            for dx in range(3):
                lhsT = wt[:, c, dx, :]
                for gi, (b0, nb) in enumerate(GROUPS):
                    nc.tensor.matmul(
                        psums[gi][:],
                        lhsT=lhsT,
                        rhs=xt[:, cc, b0 : b0 + nb, dx : dx + W + 2],
                        start=(dx == 0),
                        stop=(dx == 2),
                        skip_group_check=True,
                    )
            for gi, (b0, nb) in enumerate(GROUPS):
                ps3 = psums[gi][:].rearrange("p (b t) -> p b t", t=W + 2)
                src = ps3[:, :, 1 : W + 1]
                dst = ot[:, b0 : b0 + nb, :]
                if c % 3 == 2:
                    nc.vector.tensor_scalar_max(dst, src, 0.0)
                else:
                    nc.scalar.activation(dst, src, relu)
            nc.gpsimd.dma_start(y_ap[:, c], ot[:])


def host_weights(a, w1, w2, w3):
    """Fold the 4-way combine into one 9-tap depthwise kernel w_eff."""
    a = np.asarray(a, np.float64)
    w_eff = (
        a[1] * np.asarray(w1, np.float64)[:, 0]
        + a[2] * np.asarray(w2, np.float64)[:, 0]
        + a[3] * np.asarray(w3, np.float64)[:, 0]
    )  # [C, 3, 3]
    w_eff[:, 1, 1] += a[0]
    return w_eff.astype(np.float32)


def host_tridiag(w_eff):
    """[yi, c, dx, yo] tridiag stationary matrices: T[yi,c,dx,yo] =
    w_eff[c, yi-yo+1, dx] for |yi-yo| <= 1."""
    import ml_dtypes

    T = np.zeros((P, C, 3, P), ml_dtypes.bfloat16)
    for dy in range(3):
        yo = np.arange(max(0, 1 - dy), min(P, P + 1 - dy))
        yi = yo + dy - 1
        T[yi, :, :, yo] = w_eff[:, dy, :].astype(ml_dtypes.bfloat16)
    return T


def host_masks_scalars(w_eff):
    """One-hot diagonal masks [y, diag, yo] (diag d hits yo = yi + 1 - d)
    and per-partition-replicated scalars [y, c, dx, diag]."""
    import ml_dtypes

    dmask = np.zeros((P, 3, P), ml_dtypes.bfloat16)
    yi = np.arange(P)
    for d in range(3):
        yo = yi + 1 - d
        v = (yo >= 0) & (yo < P)
        dmask[yi[v], d, yo[v]] = 1.0
    # srep[y, c, dx, d] = w_eff[c, d, dx]
    srep = np.broadcast_to(
        w_eff.transpose(0, 2, 1)[None], (P, C, 3, 3)
    ).astype(np.float32)
    return dmask, np.ascontiguousarray(srep)


def host_inputs(x):
    """[y, c, b, w+4] zero-padded bf16, split per core along c."""
    import ml_dtypes

    xb = np.asarray(x).astype(ml_dtypes.bfloat16)  # [b, c, y, w]
    X = np.zeros((P, C, B, WP), ml_dtypes.bfloat16)
    X[:, :, :, 2 : W + 2] = xb.transpose(2, 1, 0, 3)
    return X


_PROGRAM = None


def _get_program():
    global _PROGRAM
    if _PROGRAM is None:
        nc = bacc.Bacc(
            "TRN2", target_bir_lowering=False, debug=False,
            enable_partition_id=False,
        )
        x_t = nc.dram_tensor("x", [P, CPC, B, WP], BF16, kind="ExternalInput")
        y_t = nc.dram_tensor("y", [P, CPC, B, W], BF16, kind="ExternalOutput")
        kw = {}
        if W_MODE == "host":
            w_t = nc.dram_tensor(
                "w", [P, CPC, 3, P], BF16, kind="ExternalInput"
            )
            args = (y_t.ap(), x_t.ap(), w_t.ap())
        else:
            d_t = nc.dram_tensor("dmask", [P, 3, P], BF16, kind="ExternalInput")
            s_t = nc.dram_tensor(
                "srep", [P, CPC, 3, 3], F32, kind="ExternalInput"
            )
            args = (y_t.ap(), x_t.ap(), None)
            kw = {"dmask_ap": d_t.ap(), "srep_ap": s_t.ap()}
        with TileContext(nc) as tc:
            build_tile_kernel(tc, *args, **kw)
        nc.compile()
        _PROGRAM = nc
    return _PROGRAM


def kernel(x, a, w1, w2, w3, _trace=False, _trace_kwargs=None):
    w_eff = host_weights(a, w1, w2, w3)
    X = host_inputs(x)
    in_maps = []
    if W_MODE == "host":
        T = host_tridiag(w_eff)
        for i in range(NCORES):
            cs = slice(CPC * i, CPC * (i + 1))
            in_maps.append({
                "x": np.ascontiguousarray(X[:, cs]),
                "w": np.ascontiguousarray(T[:, cs]),
            })
    else:
        dmask, srep = host_masks_scalars(w_eff)
        for i in range(NCORES):
            cs = slice(CPC * i, CPC * (i + 1))
            in_maps.append({
                "x": np.ascontiguousarray(X[:, cs]),
                "dmask": dmask,
                "srep": np.ascontiguousarray(srep[:, cs]),
            })
    nc = _get_program()
    res = bass_utils.run_bass_kernel_spmd(
        nc, in_maps, core_ids=list(range(NCORES)), trace=_trace,
        **(_trace_kwargs or {}),
    )
    # res y: [yi, cc, b, w] per core -> out[b, core*CPC+cc, y, w]
    out = np.stack(
        [np.asarray(r["y"], np.float32) for r in res.results], axis=0
    )
    out = out.transpose(3, 0, 2, 1, 4).reshape(B, C, H, W)
    if _trace:
        return out, res
    return out


# revision 8
# speedup vs baseline: 1.5625x; 1.5625x over previous
"""Depthwise 3x3 CNN combo kernel for TRN2 (8 NeuronCores, channel-parallel).

Computes  out = relu(x*a0 + dwconv(x,w1)*a1 + dwconv(x,w2)*a2 + dwconv(x,w3)*a3)
for x [8, 256, 128, 128] f32 by folding everything into a single 9-tap
depthwise conv (conv is linear in the weights; the residual a0*x is the
center tap):  w_eff = a1*w1 + a2*w2 + a3*w3,  w_eff[:,1,1] += a0.

Sharding: CHANNELS across the 8 cores (32 channels x 8 batch images per
core).  Per-core layout puts image ROWS on the partitions:

  x tile  [y=128, (c, b, w=132)]   (w padded 2 left / 2 right with zeros,
                                    host-prepadded so DMA runs are 2112B)

The vertical 3-tap conv then becomes a matmul over the partition (row)
dim with a TRIDIAGONAL stationary matrix T_dx[yi, yo] = w_eff[c, yi-yo+1, dx]
(one matrix per channel and horizontal offset dx).  Each streamed rhs
column picks up all 3 vertical taps at once (384 useful MACs/cycle vs
128 for a diagonal matmul), so the full 9-tap conv needs only 3
accumulating matmuls per psum tile:

  psum[yo, (b, t)] += sum_yi T_dx[yi, yo] * xt[yi, (b, t + dx)]   dx = 0..2

with the dx shift handled as a free-dim offset into the padded row.
Row-boundary zero padding falls out of the band truncating at the matrix
edge.  PSUM accumulates in f32; relu+downcast to bf16 runs on ScalarE
(2/3) and DVE (1/3); host upcasts.

The tridiagonal matrices are built on DVE from 3 host-supplied one-hot
diagonal masks and per-(c,dx,diag) scalars (3 small ops per matrix), so
the only HBM traffic is x in (8.7MB) and y out (8.4MB) per core.
"""

import numpy as np

import concourse.bacc as bacc
import concourse.mybir as mybir
from concourse import bass_utils
from concourse.tile import TileContext

# Problem constants (hardcoded per contract).
B = 8
C = 256
H = 128
W = 128
NCORES = 8

CPC = C // NCORES   # channels per core
P = 128             # partitions (= H rows)
WP = W + 4          # padded row width (2 zero cols each side)
GROUPS = [(0, 3), (3, 3), (6, 2)]  # batch-image groups per psum bank

F32 = mybir.dt.float32
BF16 = mybir.dt.bfloat16

# "host": DMA full tridiag matrices from HBM.  "dve": build them on-chip
# from diagonal masks + per-channel scalars.  Measured: DVE builds run at
# 1x mode (~1us/matrix, 95us total) while the 3MB host DMA costs ~9us,
# so "host" wins decisively.
W_MODE = "host"


def build_tile_kernel(tc, y_ap, x_ap, w_ap, dmask_ap=None, srep_ap=None):
    nc = tc.nc
    relu = mybir.ActivationFunctionType.Relu
    mult, add = mybir.AluOpType.mult, mybir.AluOpType.add

    with (
        tc.tile_pool(name="xpool", bufs=8) as xpool,
        tc.tile_pool(name="wpool", bufs=4) as wpool,
        tc.tile_pool(name="psum", bufs=6, space="PSUM") as psum_pool,
        tc.tile_pool(name="opool", bufs=8) as opool,
    ):
        # Input chunks of 4 channels: [y, c4, b, WP], one 8448B run per
        # partition.  All 8 chunks stay live (67KB/partition total).
        xts = []
        for g in range(CPC // 4):
            xt = xpool.tile([P, 4, B, WP], BF16, tag="xt")
            nc.sync.dma_start(xt[:], x_ap[:, 4 * g : 4 * g + 4])
            xts.append(xt)

        if W_MODE == "host":
            # Full tridiag matrices from HBM (3MB): 4 chunks of 8 channels
            # in separate tiles so early channels' matmuls only depend on
            # their own chunk's DMA.
            wts = []
            for g in range(4):
                wc = wpool.tile([P, 8, 3, P], BF16, name="wc", tag="wc")
                nc.scalar.dma_start(wc[:], w_ap[:, 8 * g : 8 * g + 8])
                wts.append(wc)
        else:
            wt = wpool.tile([P, CPC, 3, P], BF16)
            # Diagonal one-hot masks [y, diag, yo] + replicated scalars
            # [y, c, dx, diag]; build each T_dx as 1 TS + 2 STT on DVE.
            dmask = wpool.tile([P, 3, P], BF16)
            nc.scalar.dma_start(dmask[:], dmask_ap)
            srep = wpool.tile([P, CPC, 3, 3], F32)
            nc.scalar.dma_start(srep[:], srep_ap)
            for c in range(CPC):
                for dx in range(3):
                    dst = wt[:, c, dx, :]
                    nc.vector.tensor_scalar_mul(
                        dst, dmask[:, 1, :], srep[:, c, dx, 1:2]
                    )
                    nc.vector.scalar_tensor_tensor(
                        dst, dmask[:, 0, :], srep[:, c, dx, 0:1], dst,
                        mult, add,
                    )
                    nc.vector.scalar_tensor_tensor(
                        dst, dmask[:, 2, :], srep[:, c, dx, 2:3], dst,
                        mult, add,
                    )

        for c in range(CPC):
            xt = xts[c // 4]
            cc = c % 4
            ot = opool.tile([P, B, W], BF16)
            psums = [
                psum_pool.tile([P, nb * (W + 2)], F32, name="ps", tag="ps")
                for _, nb in GROUPS
            ]
            for dx in range(3):
                if W_MODE == "host":
                    lhsT = wts[c // 8][:, c % 8, dx, :]
                else:
                    lhsT = wt[:, c, dx, :]
                for gi, (b0, nb) in enumerate(GROUPS):
                    nc.tensor.matmul(
                        psums[gi][:],
                        lhsT=lhsT,
                        rhs=xt[:, cc, b0 : b0 + nb, dx : dx + W + 2],
                        start=(dx == 0),
                        stop=(dx == 2),
                        skip_group_check=True,
                    )
            for gi, (b0, nb) in enumerate(GROUPS):
                ps3 = psums[gi][:].rearrange("p (b t) -> p b t", t=W + 2)
                src = ps3[:, :, 1 : W + 1]
                dst = ot[:, b0 : b0 + nb, :]
                # Split relu+downcast evenly between ScalarE and DVE.
                if (c * len(GROUPS) + gi) % 2 == 0:
                    nc.vector.tensor_scalar_max(dst, src, 0.0)
                else:
                    nc.scalar.activation(dst, src, relu)
            # Output on the sync HWDGE queue (gpsimd DMA is SWDGE = slow).
            nc.sync.dma_start(y_ap[:, c], ot[:])


def host_weights(a, w1, w2, w3):
    """Fold the 4-way combine into one 9-tap depthwise kernel w_eff."""
    a = np.asarray(a, np.float64)
    w_eff = (
        a[1] * np.asarray(w1, np.float64)[:, 0]
        + a[2] * np.asarray(w2, np.float64)[:, 0]
        + a[3] * np.asarray(w3, np.float64)[:, 0]
    )  # [C, 3, 3]
    w_eff[:, 1, 1] += a[0]
    return w_eff.astype(np.float32)


def host_tridiag(w_eff):
    """[yi, c, dx, yo] tridiag stationary matrices: T[yi,c,dx,yo] =
    w_eff[c, yi-yo+1, dx] for |yi-yo| <= 1."""
    import ml_dtypes

    T = np.zeros((P, C, 3, P), ml_dtypes.bfloat16)
    for dy in range(3):
        yo = np.arange(max(0, 1 - dy), min(P, P + 1 - dy))
        yi = yo + dy - 1
        T[yi, :, :, yo] = w_eff[:, dy, :].astype(ml_dtypes.bfloat16)
    return T


def host_masks_scalars(w_eff):
    """One-hot diagonal masks [y, diag, yo] (diag d hits yo = yi + 1 - d)
    and per-partition-replicated scalars [y, c, dx, diag]."""
    import ml_dtypes

    dmask = np.zeros((P, 3, P), ml_dtypes.bfloat16)
    yi = np.arange(P)
    for d in range(3):
        yo = yi + 1 - d
        v = (yo >= 0) & (yo < P)
        dmask[yi[v], d, yo[v]] = 1.0
    # srep[y, c, dx, d] = w_eff[c, d, dx]
    srep = np.broadcast_to(
        w_eff.transpose(0, 2, 1)[None], (P, C, 3, 3)
    ).astype(np.float32)
    return dmask, np.ascontiguousarray(srep)


def host_inputs(x):
    """[y, c, b, w+4] zero-padded bf16, split per core along c."""
    import ml_dtypes

    xb = np.asarray(x).astype(ml_dtypes.bfloat16)  # [b, c, y, w]
    X = np.zeros((P, C, B, WP), ml_dtypes.bfloat16)
    X[:, :, :, 2 : W + 2] = xb.transpose(2, 1, 0, 3)
    return X


_PROGRAM = None


def _get_program():
    global _PROGRAM
    if _PROGRAM is None:
        nc = bacc.Bacc(
            "TRN2", target_bir_lowering=False, debug=False,
            enable_partition_id=False,
        )
        x_t = nc.dram_tensor("x", [P, CPC, B, WP], BF16, kind="ExternalInput")
        y_t = nc.dram_tensor("y", [P, CPC, B, W], BF16, kind="ExternalOutput")
        kw = {}
        if W_MODE == "host":
            w_t = nc.dram_tensor(
                "w", [P, CPC, 3, P], BF16, kind="ExternalInput"
            )
            args = (y_t.ap(), x_t.ap(), w_t.ap())
        else:
            d_t = nc.dram_tensor("dmask", [P, 3, P], BF16, kind="ExternalInput")
            s_t = nc.dram_tensor(
                "srep", [P, CPC, 3, 3], F32, kind="ExternalInput"
            )
            args = (y_t.ap(), x_t.ap(), None)
            kw = {"dmask_ap": d_t.ap(), "srep_ap": s_t.ap()}
        with TileContext(nc) as tc:
            build_tile_kernel(tc, *args, **kw)
        nc.compile()
        _PROGRAM = nc
    return _PROGRAM


def kernel(x, a, w1, w2, w3, _trace=False, _trace_kwargs=None):
    w_eff = host_weights(a, w1, w2, w3)
    X = host_inputs(x)
    in_maps = []
    if W_MODE == "host":
        T = host_tridiag(w_eff)
        for i in range(NCORES):
            cs = slice(CPC * i, CPC * (i + 1))
            in_maps.append({
                "x": np.ascontiguousarray(X[:, cs]),
                "w": np.ascontiguousarray(T[:, cs]),
            })
    else:
        dmask, srep = host_masks_scalars(w_eff)
        for i in range(NCORES):
            cs = slice(CPC * i, CPC * (i + 1))
            in_maps.append({
                "x": np.ascontiguousarray(X[:, cs]),
                "dmask": dmask,
                "srep": np.ascontiguousarray(srep[:, cs]),
            })
    nc = _get_program()
    res = bass_utils.run_bass_kernel_spmd(
        nc, in_maps, core_ids=list(range(NCORES)), trace=_trace,
        **(_trace_kwargs or {}),
    )
    # res y: [yi, cc, b, w] per core -> out[b, core*CPC+cc, y, w]
    out = np.stack(
        [np.asarray(r["y"], np.float32) for r in res.results], axis=0
    )
    out = out.transpose(3, 0, 2, 1, 4).reshape(B, C, H, W)
    if _trace:
        return out, res
    return out
